# revision 24
# baseline (speedup 1.0000x reference)
"""TRN2 Bass kernel for CompressedLinearLayer: out = x @ (A @ B.T).T + bias.

Computed low-rank: t = x @ B  (rank 512), out = t @ A.T + bias.
Sharding: data-parallel over the 8192 rows of x (1024 rows per core);
B, A.T, bias replicated. No collectives.

v4: stage-1 half in fp8 (e4m3) with DoubleRow perf mode (2x PE
throughput).  The first K8=2048 rows of d_in are quantized on host:
x*0.25 and B*4 (compensated power-of-2 scales keep products unscaled,
so fp8 DoubleRow and bf16 matmuls accumulate into the SAME PSUM).
Measured rel err 1.78e-2 at K8=1792 against the 2e-2 gate; K8=2048
predicts 1.90e-2 (bf16-only baseline was 2.9e-3).

Lessons baked in from v3 traces:
- the gpsimd DMA queue delivers only ~52 B/ns (vs ~180 per hw ring),
  so it carries only late-needed tiles (bias, bg1, bg3, at3); all
  PE-critical tiles ride sync+scalar in strict need order.
- aggregate DMA tops out ~370 B/ns and ramps up over the first ~15us;
  the input stream is supply-limited until ~24us, so the head stays
  fine-grained (first fp8 pair split into 64KB pieces).
- stage-1 and stage-2 share ONE PSUM pool (8 bufs of [128,512]): a
  separate stage-2 pool serialized on the pool-close barrier behind
  the whole tT evacuation chain (3.5us PE stall).
- tT evacuation casts alternate DVE / Activation engines so the
  per-cast 680ns chain keeps up with the mc-major matmul tail.
- end-of-program event teardown is ~fixed (~57 events x ~150ns per
  engine) regardless of DMA count; don't bother consolidating DMAs.

Layout (per core):
  xT8  [2048, 1024] fp8e4 = (x rows shard).T[0:2048]   * 0.25
  xTb  [2048, 1024] bf16  = (x rows shard).T[2048:]
  b8   [2048, 512]  fp8e4 = B[0:2048] * 4
  bb   [2048, 512]  bf16  = B[2048:]
  at   [512, 4096]  bf16  A.T
  bias [4096]       f32
  out  [1024, 4096] bf16  (upcast to f32 on host)
"""
import numpy as np
import ml_dtypes

import concourse.bacc as bacc
import concourse.mybir as mybir
import concourse.tile as tile
from concourse.bass_utils import run_bass_kernel_spmd

N_CORES = 8
BATCH, SEQ = 4, 2048
D_IN, D_OUT, RANK = 4096, 4096, 512
ROWS_TOTAL = BATCH * SEQ           # 8192
ROWS = ROWS_TOTAL // N_CORES       # 1024 rows per core

F32 = mybir.dt.float32
BF16 = mybir.dt.bfloat16
F8 = mybir.dt.float8e4
DR = mybir.MatmulPerfMode.DoubleRow

NPAIR = 8                 # fp8 DoubleRow pairs (256 d_in rows each)
K8 = NPAIR * 256          # 2048 fp8 rows of d_in
KBF = (D_IN - K8) // 128  # 16 bf16 k-chunks
SX = 0.25                 # host scale on x fp8 region
SB = 4.0                  # host scale on B fp8 region (SX*SB == 1)

RC = RANK // 128     # 4 rank chunks
NBLK = 2             # row blocks per core
BROWS = ROWS // NBLK # 512 rows per block
MB2 = BROWS // 128   # 4 row chunks of 128 per block

_compiled = {}


def _build():
    nc = bacc.Bacc("TRN2", target_bir_lowering=False, debug=False)

    xT8_d = nc.declare_dram_parameter("xT8", [K8, ROWS], F8, isOutput=False)
    xTb_d = nc.declare_dram_parameter("xTb", [D_IN - K8, ROWS], BF16,
                                      isOutput=False)
    b8_d = nc.declare_dram_parameter("b8", [K8, RANK], F8, isOutput=False)
    bb_d = nc.declare_dram_parameter("bb", [D_IN - K8, RANK], BF16,
                                     isOutput=False)
    at_d = nc.declare_dram_parameter("at", [RANK, D_OUT], BF16, isOutput=False)
    bias_d = nc.declare_dram_parameter("bias", [D_OUT], F32, isOutput=False)
    out_d = nc.declare_dram_parameter("out", [ROWS, D_OUT], BF16, isOutput=True)

    rings = [nc.sync, nc.scalar]

    with tile.TileContext(nc) as tc:
        with (
            tc.tile_pool(name="wb", bufs=1) as wb,
            tc.tile_pool(name="op", bufs=3) as op,
            tc.tile_pool(name="ps", bufs=8, space="PSUM") as psp,
        ):
            bias_bc = wb.tile([128, D_OUT], F32, tag="bias_bc")
            warm_in = wb.tile([128, 640], BF16, tag="warm_in", name="warm_in")

            # fp8 x tiles: pair0 blk0 split in row halves for a fast start.
            # All blk0 tiles stream (and are consumed) before any blk1 tile:
            # the first ~16us are DMA-supply-limited, so early demand is
            # halved by deferring block 1.
            x8p0b0 = [wb.tile([128, 2, 256], F8, tag=f"x8p0b0{h}",
                              name=f"x8p0b0{h}") for h in range(2)]
            x8 = {}
            for key, np_, blks in (("p0", 1, (1,)), ("p1", 1, (0, 1)),
                                   ("p23", 2, (0, 1)), ("p45", 2, (0,)),
                                   ("p67", 2, (0,)), ("p4567", 4, (1,))):
                for blk in blks:
                    x8[key, blk] = wb.tile([128, 2 * np_, BROWS], F8,
                                           tag=f"x8{key}_{blk}",
                                           name=f"x8{key}_{blk}")
            # b8 pair0 split by rank halves (mc 0-1 / mc 2-3)
            b8p0 = [wb.tile([128, 2, 256], F8, tag=f"b8p0{h}",
                            name=f"b8p0{h}") for h in range(2)]
            b8 = {}
            for key, np_ in (("p1", 1), ("p23", 2), ("p45", 2), ("p67", 2)):
                b8[key] = wb.tile([128, 2 * np_, RANK], F8, tag=f"b8{key}",
                                  name=f"b8{key}")

            # bf16 region: 16 k-chunks as 8 half-groups of 2 (one DMA each,
            # spread across both rings so each lands in half the time)
            xgh = [[wb.tile([128, 2, ROWS], BF16, tag=f"xg{i}_{h}",
                            name=f"xg{i}_{h}") for h in range(2)]
                   for i in range(4)]
            bg = [wb.tile([128, 4, RANK], BF16, tag=f"bg{i}", name=f"bg{i}")
                  for i in range(4)]
            at_sb = [wb.tile([128, D_OUT], BF16, tag=f"at{r}", name=f"at{r}")
                     for r in range(RC)]
            tT = [[wb.tile([128, BROWS], BF16, tag=f"tT{b}_{r}",
                           name=f"tT{b}_{r}") for r in range(RC)]
                  for b in range(NBLK)]

            # PE clock warm-up (DVFS ~0.8->2.4GHz): dummy matmuls gated only
            # on the DVE memzero of the scratch (the tile framework requires
            # tiles to be written before read).  Widths shrink toward the
            # end to limit overshoot past first-data arrival.
            nc.vector.memzero(warm_in[:])
            ps_warm = psp.tile([128, BROWS], F32, tag="ps", name="warm")
            for wcols in (512, 512, 256, 256, 256, 128, 128, 128):
                nc.tensor.matmul(
                    ps_warm[:, 0:wcols], warm_in[:, 0:128],
                    warm_in[:, 128:128 + wcols],
                    start=True, stop=True,
                )

            # ---- DMA streams: strict need order per ring ----
            def r8(dram, r0, r1, c0=None, c1=None):
                sl = dram[r0:r1, :] if c0 is None else dram[r0:r1, c0:c1]
                return sl.rearrange("(ks p) m -> p ks m", p=128)

            def xgh_src(i, h):
                c0 = (4 * i + 2 * h) * 128
                return r8(xTb_d, c0, c0 + 256)

            # sync ring
            nc.sync.dma_start(b8p0[0][:], b8_d[0:256, 0:256]
                              .rearrange("(ks p) r -> p ks r", p=128))
            nc.sync.dma_start(x8p0b0[1][:], r8(xT8_d, 0, 256, 256, 512))
            nc.sync.dma_start(b8["p1"][:], r8(b8_d, 256, 512))
            nc.sync.dma_start(x8["p23", 0][:], r8(xT8_d, 512, 1024, 0, BROWS))
            nc.sync.dma_start(b8["p45"][:], r8(b8_d, 1024, 1536))
            nc.sync.dma_start(b8["p67"][:], r8(b8_d, 1536, 2048))
            nc.sync.dma_start(x8["p0", 1][:], r8(xT8_d, 0, 256, BROWS, ROWS))
            nc.sync.dma_start(x8["p23", 1][:],
                              r8(xT8_d, 512, 1024, BROWS, ROWS))
            nc.sync.dma_start(xgh[0][0][:], xgh_src(0, 0))
            nc.sync.dma_start(bg[1][:], r8(bb_d, 512, 1024))
            nc.sync.dma_start(xgh[1][1][:], xgh_src(1, 1))
            nc.sync.dma_start(bg[2][:], r8(bb_d, 1024, 1536))
            nc.sync.dma_start(xgh[2][1][:], xgh_src(2, 1))
            nc.sync.dma_start(xgh[3][0][:], xgh_src(3, 0))
            nc.sync.dma_start(at_sb[0][:], at_d[0:128, :])
            nc.sync.dma_start(at_sb[2][:], at_d[256:384, :])
            # scalar ring
            nc.scalar.dma_start(x8p0b0[0][:], r8(xT8_d, 0, 256, 0, 256))
            nc.scalar.dma_start(b8p0[1][:], b8_d[0:256, 256:512]
                                .rearrange("(ks p) r -> p ks r", p=128))
            nc.scalar.dma_start(x8["p1", 0][:], r8(xT8_d, 256, 512, 0, BROWS))
            nc.scalar.dma_start(b8["p23"][:], r8(b8_d, 512, 1024))
            nc.scalar.dma_start(x8["p45", 0][:],
                                r8(xT8_d, 1024, 1536, 0, BROWS))
            nc.scalar.dma_start(x8["p67", 0][:],
                                r8(xT8_d, 1536, 2048, 0, BROWS))
            nc.scalar.dma_start(x8["p1", 1][:],
                                r8(xT8_d, 256, 512, BROWS, ROWS))
            nc.scalar.dma_start(x8["p4567", 1][:],
                                r8(xT8_d, 1024, 2048, BROWS, ROWS))
            nc.scalar.dma_start(bg[0][:], r8(bb_d, 0, 512))
            nc.scalar.dma_start(xgh[0][1][:], xgh_src(0, 1))
            nc.scalar.dma_start(xgh[1][0][:], xgh_src(1, 0))
            nc.scalar.dma_start(xgh[2][0][:], xgh_src(2, 0))
            nc.scalar.dma_start(bg[3][:], r8(bb_d, 1536, 2048))
            nc.scalar.dma_start(xgh[3][1][:], xgh_src(3, 1))
            nc.scalar.dma_start(at_sb[1][:], at_d[128:256, :])
            nc.scalar.dma_start(at_sb[3][:], at_d[384:512, :])
            # gpsimd queue (slow ~52 B/ns): only the tiny bias load
            nc.gpsimd.dma_start(bias_bc[0:1, :], bias_d[None, :])
            nc.gpsimd.partition_broadcast(bias_bc[:], bias_bc[0:1, :])

            ps1 = [[psp.tile([128, BROWS], F32, tag="ps",
                             name=f"ps1_{blk}_{i}") for i in range(RC)]
                   for blk in range(NBLK)]

            # ---- stage 1: t[rank, rows] = B.T @ x ----
            def filler(n, wcols=256):
                # DVFS-keepalive in known DMA-starve windows.  Writes go to
                # ps_warm (bank shared with ps1[1][3]), so fillers are only
                # legal BEFORE the first blk-1 matmul: the tensor engine is
                # serial, hence no race while only blk-0 psums accumulate.
                for _ in range(n):
                    nc.tensor.matmul(
                        ps_warm[:, 0:wcols], warm_in[:, 0:128],
                        warm_in[:, 128:128 + wcols],
                        start=True, stop=True,
                    )

            # block 0 of all fp8 pairs first (fine pieces at the head)
            for h in range(2):      # row halves of pair0 block 0
                for mc in range(RC):
                    # start=True marks the WHOLE 2KB psum bank pending-zero,
                    # so only the first partial write may set it; the second
                    # half accumulates into the already-zeroed region
                    nc.tensor.matmul(
                        ps1[0][mc][:, h * 256:(h + 1) * 256],
                        b8p0[mc // 2][:, :, (mc % 2) * 128:(mc % 2 + 1) * 128],
                        x8p0b0[h][:],
                        start=(h == 0), stop=False, perf_mode=DR,
                    )

            def pair_mm(key, j, blk, xkey=None, xj=None, start=False):
                xj = j if xj is None else xj
                for mc in range(RC):
                    nc.tensor.matmul(
                        ps1[blk][mc][:],
                        b8[key][:, 2 * j:2 * j + 2, mc * 128:(mc + 1) * 128],
                        x8[xkey or key, blk][:, 2 * xj:2 * xj + 2, :],
                        start=start, stop=False, perf_mode=DR,
                    )

            pair_mm("p1", 0, 0)
            filler(1)
            pair_mm("p23", 0, 0)
            pair_mm("p23", 1, 0)
            filler(2)
            pair_mm("p45", 0, 0)
            pair_mm("p45", 1, 0)
            filler(1)
            pair_mm("p67", 0, 0)
            pair_mm("p67", 1, 0)
            filler(1)
            # block 1 of all fp8 pairs (no fillers past this point: ps_warm
            # shares its PSUM bank with ps1[1][3])
            for mc in range(RC):
                nc.tensor.matmul(
                    ps1[1][mc][:],
                    b8p0[mc // 2][:, :, (mc % 2) * 128:(mc % 2 + 1) * 128],
                    x8["p0", 1][:],
                    start=True, stop=False, perf_mode=DR,
                )
            pair_mm("p1", 0, 1)
            pair_mm("p23", 0, 1)
            pair_mm("p23", 1, 1)
            pair_mm("p45", 0, 1, xkey="p4567", xj=0)
            pair_mm("p45", 1, 1, xkey="p4567", xj=1)
            pair_mm("p67", 0, 1, xkey="p4567", xj=2)
            pair_mm("p67", 1, 1, xkey="p4567", xj=3)

            def bf_rhs(c, blk):
                return xgh[c // 4][(c % 4) // 2][:, c % 2,
                                                 blk * BROWS:(blk + 1) * BROWS]

            # bf16 chunks, block-interleaved
            for c in range(KBF - 1):
                for blk in range(NBLK):
                    for mc in range(RC):
                        nc.tensor.matmul(
                            ps1[blk][mc][:],
                            bg[c // 4][:, c % 4, mc * 128:(mc + 1) * 128],
                            bf_rhs(c, blk),
                            start=False, stop=False,
                        )
            # last chunk mc-major; evacuation casts alternate DVE /
            # Activation so the copies keep pace with the matmul tail
            c = KBF - 1
            for blk in range(NBLK):
                for mc in range(RC):
                    nc.tensor.matmul(
                        ps1[blk][mc][:],
                        bg[c // 4][:, c % 4, mc * 128:(mc + 1) * 128],
                        bf_rhs(c, blk),
                        start=False, stop=True,
                    )
                    if (blk * RC + mc) % 2 == 0:
                        nc.vector.tensor_copy(tT[blk][mc][:], ps1[blk][mc][:])
                    else:
                        nc.scalar.copy(tT[blk][mc][:], ps1[blk][mc][:])

            # ---- stage 2: out[rows, dout] = t.T @ A.T + bias ----
            # same PSUM pool: each new [128,512] psum tile only waits for
            # the previous tenant of its bank (no pool-close barrier)
            units = [(blk, rc2) for blk in range(NBLK) for rc2 in range(MB2)]
            for ui, (blk, rc2) in enumerate(units):
                last = ui == len(units) - 1
                row0 = rc2 * 128
                ot = op.tile([128, D_OUT], BF16, tag="ot",
                             name=f"ot{blk}_{rc2}")
                for sc in range(8):          # 512-wide column sub-units
                    d0 = sc * 512
                    fine = last and sc >= 4
                    ps2 = psp.tile([128, 512], F32, tag="ps",
                                   name=f"ps2_{blk}_{rc2}_{sc}")
                    # (A 256-wide column split of the final sub-unit was
                    # tried to shorten the drain: it cut last-store lag
                    # 2.5->2.1us but reproducibly cost a ~0.7us PE gap
                    # right before the final matmuls -- net loss.)
                    for k in range(RC):
                        nc.tensor.matmul(
                            ps2[:],
                            tT[blk][k][:, row0:row0 + 128],
                            at_sb[k][:, d0:d0 + 512],
                            start=(k == 0),
                            stop=(k == RC - 1),
                        )
                    nc.vector.tensor_add(
                        ot[:, d0:d0 + 512], ps2[:], bias_bc[:, d0:d0 + 512],
                    )
                    if fine:
                        # final row chunk: 0.125MB stores right after each
                        # bias-add so the drain tail stays short (finer
                        # 256-wide splits measured SLOWER: ~420ns DVE-add
                        # overhead and ~600ns per store-descriptor issue
                        # dominate at that grain)
                        rings[sc % 2].dma_start(
                            out_d[blk * BROWS + row0:blk * BROWS + row0 + 128,
                                  d0:d0 + 512],
                            ot[:, d0:d0 + 512],
                        )
                    elif last and sc == 3:
                        # coarse half of the final row chunk: store as soon
                        # as its last column is ready, ahead of the fine ones
                        rings[0].dma_start(
                            out_d[blk * BROWS + row0:blk * BROWS + row0 + 128,
                                  0:2048],
                            ot[:, 0:2048],
                        )
                if not last:
                    rings[ui % 2].dma_start(
                        out_d[blk * BROWS + row0:blk * BROWS + row0 + 128, :],
                        ot[:],
                    )

    nc.compile()
    return nc


def _get_nc():
    if "nc" not in _compiled:
        _compiled["nc"] = _build()
    return _compiled["nc"]


def run(inputs, trace=False, trace_kwargs=None):
    """Shard, execute on 8 cores, gather. Returns (output, BassKernelResults)."""
    x = np.asarray(inputs["x"], dtype=np.float32)
    A = np.asarray(inputs["A"], dtype=np.float32)
    B = np.asarray(inputs["B"], dtype=np.float32)
    bias = np.asarray(inputs["bias"], dtype=np.float32)

    x_flat = x.reshape(ROWS_TOTAL, D_IN)
    f8 = ml_dtypes.float8_e4m3
    b8 = (B[:K8] * SB).astype(f8)
    bb = B[K8:].astype(ml_dtypes.bfloat16)
    AT_bf = np.ascontiguousarray(A.T).astype(ml_dtypes.bfloat16)
    in_maps = []
    for i in range(N_CORES):
        xT_i = np.ascontiguousarray(x_flat[i * ROWS:(i + 1) * ROWS].T)
        in_maps.append({
            "xT8": (xT_i[:K8] * SX).astype(f8),
            "xTb": xT_i[K8:].astype(ml_dtypes.bfloat16),
            "b8": b8,
            "bb": bb,
            "at": AT_bf,
            "bias": bias,
        })

    nc = _get_nc()
    kwargs = {}
    if trace:
        kwargs["trace"] = True
        kwargs["trace_kwargs"] = trace_kwargs or {}
    res = None
    for attempt in range(3):
        try:
            res = run_bass_kernel_spmd(
                nc, in_maps, core_ids=list(range(N_CORES)), **kwargs
            )
        except Exception:
            # transient device/runtime hiccup; retry
            if attempt == 2:
                raise
            continue
        out = np.concatenate(
            [np.asarray(res.results[i]["out"]).astype(np.float32)
             for i in range(N_CORES)],
            axis=0,
        )
        if np.isfinite(out).all():
            return out.reshape(BATCH, SEQ, D_OUT), res
    return out.reshape(BATCH, SEQ, D_OUT), res


def kernel(**inputs) -> np.ndarray:
    out, _ = run(inputs)
    return out
